# revision 22
# baseline (speedup 1.0000x reference)
"""Linformer text encoder on 8 TRN2 NeuronCores.

Sharding: pure data-parallel over batch (32 seqs -> 4 per core), weights
replicated, no collectives. Host does the embedding gather and folds LN
gamma into the following projection weights (exact math), plus computes
layer-0 LN stats. Device does everything else in bf16 matmuls with f32
accumulation.

Engine plan (v2):
  - ScalarE: softmax Exp + psum->sbuf copies (COPY needs no act table) +
    per-phase batched LN stat conversion (Ln/Exp loaded once per phase).
  - GpSimd: LN affine applies (sbuf f32 -> sbuf bf16).
  - DVE: bn_stats, softmax reduce/recip/broadcast-mul, residual adds,
    transpose psum->sbuf copies.
  - PE: matmuls + transposes, software-pipelined so softmax latency is
    hidden (keeps HAM at full clock).
  - Final token-mean folded into layer 3 (ones-matmul accumulation in
    attention pass + pooled-W2 trick in FF), so no final re-read of Xp.

Self-contained: hardcodes all shapes from the problem spec.
"""

import sys

sys.path.insert(0, "/opt/trn_rl_repo")

from contextlib import ExitStack
from dataclasses import dataclass

import ml_dtypes
import numpy as np

import concourse.bass as bass
import concourse.tile as tile
from concourse import bacc, mybir
from concourse.bass_utils import run_bass_kernel_spmd
from concourse.masks import make_identity

F32 = mybir.dt.float32
BF16 = mybir.dt.bfloat16
AF = mybir.ActivationFunctionType
ALU = mybir.AluOpType
AX = mybir.AxisListType

EPS = 1e-5


@dataclass(frozen=True)
class Dims:
    B_loc: int = 4      # sequences per core
    N: int = 2048       # tokens per sequence
    D: int = 1024
    H: int = 16
    DH: int = 64
    K: int = 64
    FF: int = 4096
    L: int = 4

    @property
    def R(self):
        return self.B_loc * self.N

    @property
    def n_blk(self):        # 128-token blocks per sequence
        return self.N // 128

    @property
    def n_chunk(self):      # 512-token chunks per sequence
        return self.N // 512


def build(dims: Dims, n_cores: int, biases, dbg=False):
    """Emit the full per-core program. `biases` is a dict of host numpy
    vectors (cq, bu, bo, bz per layer) or None entries when zero."""
    d = dims
    nc = bacc.Bacc("TRN2", target_bir_lowering=False, debug=False,
                   num_devices=n_cores, enable_asserts=False)

    x0 = nc.dram_tensor("x0", [d.R, d.D], F32, kind="ExternalInput")
    s1_0 = nc.dram_tensor("s1_0", [128, d.R // 128, 2], F32, kind="ExternalInput")
    wq_d = [nc.dram_tensor(f"wq{l}", [d.D, d.D], BF16, kind="ExternalInput") for l in range(d.L)]
    wk_d = [nc.dram_tensor(f"wk{l}", [d.D, d.DH], BF16, kind="ExternalInput") for l in range(d.L)]
    pk_d = [nc.dram_tensor(f"pk{l}", [d.N, d.K], BF16, kind="ExternalInput") for l in range(d.L)]
    wo_d = [nc.dram_tensor(f"wo{l}", [d.D, d.D], BF16, kind="ExternalInput") for l in range(d.L)]
    w1_d = [nc.dram_tensor(f"w1{l}", [d.D, d.FF], BF16, kind="ExternalInput") for l in range(d.L)]
    w2_d = [nc.dram_tensor(f"w2{l}", [d.FF, d.D], BF16, kind="ExternalInput") for l in range(d.L)]
    lnfg = nc.dram_tensor("lnfg", [1, d.D], F32, kind="ExternalInput")
    lnfb = nc.dram_tensor("lnfb", [1, d.D], F32, kind="ExternalInput")
    out = nc.dram_tensor("out", [d.B_loc, d.D], F32, kind="ExternalOutput")

    bias_d = {}
    for l in range(d.L):
        for nm in ("cq", "bu", "bo", "bz"):
            if biases and biases.get((nm, l)) is not None:
                arr = biases[(nm, l)]
                bias_d[(nm, l)] = nc.dram_tensor(
                    f"{nm}{l}", list(arr.shape), BF16, kind="ExternalInput")

    ikind = "ExternalOutput" if dbg else "Internal"
    Xp = nc.dram_tensor("Xp", [d.R, d.D], F32, kind=ikind)   # post-attention residual
    Xr = nc.dram_tensor("Xr", [d.R, d.D], F32, kind=ikind)   # post-FF residual
    if dbg:
        Hd = nc.dram_tensor("Hd", [d.R, d.D], BF16, kind="ExternalOutput")
        KVd = nc.dram_tensor("KVd", [2 * d.DH, d.K], BF16, kind="ExternalOutput")
        Qd = nc.dram_tensor("Qd", [d.D, 512], BF16, kind="ExternalOutput")
        Ad = nc.dram_tensor("Ad", [128, d.H * d.K], BF16, kind="ExternalOutput")
        Od = nc.dram_tensor("Od", [d.D, 128], BF16, kind="ExternalOutput")
    Es = nc.dram_tensor("Es", [d.B_loc, 2, 512], F32)  # l3 token-sums of Xp
    Zs = nc.dram_tensor("Zs", [d.B_loc, 2, 512], F32)  # l3 pooled FF delta

    nD = d.D // 128       # 8 feature chunks
    nF = d.FF // 128      # 32 ff chunks
    nblk = d.R // 128     # 64 token blocks per core

    with ExitStack() as ctx:
        tc = ctx.enter_context(tile.TileContext(nc))
        const = ctx.enter_context(tc.tile_pool(name="const", bufs=1))
        small = ctx.enter_context(tc.tile_pool(name="small", bufs=4))
        stats = ctx.enter_context(tc.tile_pool(name="stats", bufs=1))

        idt = const.tile([128, 128], BF16)
        make_identity(nc, idt[:])
        ones_bf = const.tile([1, 512], BF16)
        nc.vector.memset(ones_bf[:], 1.0)
        ones_bc = const.tile([128, 1], BF16)
        nc.vector.memset(ones_bc[:], 1.0)
        eps_t = const.tile([128, 1], F32)
        nc.vector.memset(eps_t[:], EPS)
        ilo = const.tile([64, 128], BF16)
        nc.vector.memset(ilo[:], 0.0)
        make_identity(nc, ilo[:, 0:64])
        ihi = const.tile([64, 128], BF16)
        nc.vector.memset(ihi[:], 0.0)
        make_identity(nc, ihi[:, 64:128])

        # persistent stat tiles per layer: s_in (LN1 stats of layer input),
        # s_mid (LN2 stats of post-attn residual)
        s_in = [stats.tile([128, nblk, 2], F32, name=f"sin{l}") for l in range(d.L)]
        s_mid = [stats.tile([128, nblk, 2], F32, name=f"smid{l}") for l in range(d.L)]
        nc.sync.dma_start(s_in[0][:], s1_0.ap())

        def bn_mv(xt_ap, out_mv):
            """bn stats of [128, D] f32 -> out_mv [128, 2] = (mean, var)."""
            ngrp = xt_ap.shape[1] // 512
            bns = small.tile([128, ngrp, 6], F32, tag="bns")
            for g in range(ngrp):
                nc.vector.bn_stats(bns[:, g, :], xt_ap[:, g * 512:(g + 1) * 512])
            nc.vector.bn_aggr(out_mv, bns[:])

        def convert_stats(src, rstd, nmr):
            """Batched (mean,var)[128, nblk, 2] -> rstd/nmr [128, nblk].
            Two act-table loads total (Ln, Exp)."""
            lnv = small.tile([128, nblk], F32, tag="lnv", bufs=1)
            nc.scalar.activation(lnv[:], src[:, :, 1], AF.Ln, bias=eps_t[:])
            nc.scalar.activation(rstd[:], lnv[:], AF.Exp, scale=-0.5)
            nc.vector.scalar_tensor_tensor(nmr[:], src[:, :, 0], -1.0, rstd[:],
                                           ALU.mult, ALU.mult)

        def transpose_into(h_ap, dst_tile, dst_chunk0, tcol, ptr):
            """Transpose h_ap [128, nch*128]: chunk c -> dst[:, dst_chunk0+c,
            tcol*128:+128]. PE transposes + DVE copies."""
            nch = h_ap.shape[1] // 128
            for c0 in range(0, nch, 4):
                cn = min(4, nch - c0)
                pt = ptr.tile([128, 512], BF16, tag="pt")
                for c in range(cn):
                    nc.tensor.transpose(
                        pt[:, c * 128:(c + 1) * 128],
                        h_ap[:, (c0 + c) * 128:(c0 + c + 1) * 128], idt[:])
                nc.vector.tensor_copy(
                    dst_tile[:, dst_chunk0 + c0:dst_chunk0 + c0 + cn,
                             tcol * 128:(tcol + 1) * 128],
                    pt[:, :cn * 128].rearrange("p (a f) -> p a f", a=cn))

        def load_bias_rows(pool, l, names):
            out = {}
            for nm in names:
                if (nm, l) in bias_d:
                    dram = bias_d[(nm, l)]
                    t = pool.tile([1, dram.shape[0]], BF16, tag=f"b{nm}",
                                  name=f"b{nm}{l}")
                    nc.sync.dma_start(t[:], dram.ap()[None, :])
                    out[(nm, l)] = t
            return out

        for l in range(d.L):
            src = x0 if l == 0 else Xr
            last = l == d.L - 1

            # =================== attention phase =======================
            with tc.tile_pool(name=f"wat{l}", bufs=1) as wat, \
                 tc.tile_pool(name=f"pha{l}", bufs=2) as htp, \
                 tc.tile_pool(name=f"wka{l}", bufs=2) as work, \
                 tc.tile_pool(name=f"xa{l}", bufs=3) as xin, \
                 tc.tile_pool(name=f"xba{l}", bufs=3) as xbin, \
                 tc.tile_pool(name=f"ha{l}", bufs=2) as hbuf, \
                 tc.tile_pool(name=f"oa{l}", bufs=3) as outp, \
                 tc.tile_pool(name=f"pmm{l}", bufs=5, space="PSUM") as pmm, \
                 tc.tile_pool(name=f"ptr{l}", bufs=2, space="PSUM") as ptr, \
                 tc.tile_pool(name=f"pkv{l}", bufs=1, space="PSUM") as pkv:
                rstd1 = stats.tile([128, nblk], F32, tag="rstd1", name=f"rstd1_{l}")
                nmr1 = stats.tile([128, nblk], F32, tag="nmr1", name=f"nmr1_{l}")
                convert_stats(s_in[l], rstd1, nmr1)

                wqS = wat.tile([128, nD, d.D], BF16, tag="wq")
                nc.sync.dma_start(wqS[:], wq_d[l].ap().rearrange("(a p) n -> p a n", p=128))
                wkS = wat.tile([128, nD, d.DH], BF16, tag="wk")
                nc.sync.dma_start(wkS[:], wk_d[l].ap().rearrange("(a p) n -> p a n", p=128))
                pkS = wat.tile([128, d.n_blk, d.K], BF16, tag="pk")
                nc.sync.dma_start(pkS[:], pk_d[l].ap().rearrange("(a p) k -> p a k", p=128))
                woS = wat.tile([128, nD, d.D], BF16, tag="wo")
                nc.sync.dma_start(woS[:], wo_d[l].ap().rearrange("(a p) n -> p a n", p=128))
                bias_sb = load_bias_rows(wat, l, ("cq", "bo"))

                hTs = {}
                cur_kv = {}
                cur_bd = {}
                xt_pre = {}

                def prefetch_xt(r):
                    t = xin.tile([128, d.D], F32, tag="xt")
                    nc.sync.dma_start(t[:], src.ap()[r * 128:(r + 1) * 128, :])
                    xt_pre[r] = t

                def pass_a_block(b, t):
                    r = b * d.n_blk + t
                    if t == 0:
                        hTs[b] = htp.tile([128, nD, d.N], BF16, tag="hT",
                                          name=f"hT{l}_{b}")
                        kvp = pkv.tile([128, 64], F32, tag="kvp", name=f"kvp{l}_{b}")
                        cur_kv[b] = (kvp[0:64, :], kvp[64:128, :])
                    hT = hTs[b]
                    kvT_ps, kv_ps = cur_kv[b]
                    if r not in xt_pre:
                        prefetch_xt(r)
                    xt = xt_pre.pop(r)
                    h = hbuf.tile([128, d.D], BF16, tag="h")
                    nc.gpsimd.tensor_scalar(h[:], xt[:], rstd1[:, r:r + 1],
                                            nmr1[:, r:r + 1], ALU.mult, ALU.add)
                    if dbg and l == 0:
                        nc.sync.dma_start(Hd.ap()[r * 128:(r + 1) * 128, :], h[:])
                    transpose_into(h[:], hT, 0, t, ptr)
                    hk_ps = pmm.tile([128, d.DH], F32, tag="mm", name=f"hkps{l}_{r}")
                    for dc in range(nD):
                        nc.tensor.matmul(
                            hk_ps[:], hT[:, dc, t * 128:(t + 1) * 128],
                            wkS[:, dc, :], start=(dc == 0), stop=(dc == nD - 1))
                    hk = work.tile([128, d.DH], BF16, tag="hk")
                    nc.scalar.copy(hk[:], hk_ps[:])
                    nc.tensor.matmul(kvT_ps[:], hk[:], pkS[:, t, :],
                                     start=(t == 0), stop=(t == d.n_blk - 1))
                    nc.tensor.matmul(kv_ps[:], pkS[:, t, :], hk[:],
                                     start=(t == 0), stop=(t == d.n_blk - 1))
                    # prefetch next pass-A xt up to two blocks ahead
                    for dt in (1, 2):
                        if t + dt < d.n_blk and (r + dt) not in xt_pre:
                            prefetch_xt(r + dt)

                def bd_build(b):
                    kvT_ps, kv_ps = cur_kv.pop(b)
                    kvT = work.tile([64, 64], BF16, tag="kvT")
                    kv = work.tile([64, 64], BF16, tag="kv")
                    nc.scalar.copy(kvT[:], kvT_ps[:])
                    nc.scalar.copy(kv[:], kv_ps[:])
                    if dbg and l == 0 and b == 0:
                        nc.sync.dma_start(KVd.ap()[0:d.DH, :], kvT[:])
                        nc.sync.dma_start(KVd.ap()[d.DH:2 * d.DH, :], kv[:])
                    bdT_ps = pmm.tile([128, 128], F32, tag="mm", name="bdTps")
                    nc.tensor.matmul(bdT_ps[:, 0:64], ilo[:], kvT[:])
                    nc.tensor.matmul(bdT_ps[:, 64:128], ihi[:], kvT[:])
                    bdv_ps = pmm.tile([128, 128], F32, tag="mm", name="bdvps")
                    nc.tensor.matmul(bdv_ps[:, 0:64], ilo[:], kv[:])
                    nc.tensor.matmul(bdv_ps[:, 64:128], ihi[:], kv[:])
                    bdT = work.tile([128, 128], BF16, tag="bdT")
                    nc.scalar.copy(bdT[:], bdT_ps[:])
                    bdv = work.tile([128, 128], BF16, tag="bdv")
                    nc.scalar.copy(bdv[:], bdv_ps[:])
                    cur_bd[b] = (bdT, bdv)

                # ---- sequence-0 pass A, standalone ----
                prefetch_xt(0)
                for t in range(d.n_blk):
                    pass_a_block(0, t)
                bd_build(0)

                for b in range(d.B_loc):
                    hT = hTs.pop(b)
                    bdT, bdv = cur_bd.pop(b)
                    nxt = b + 1 if b + 1 < d.B_loc else None
                    if nxt is not None:
                        prefetch_xt(nxt * d.n_blk)

                    # state of the software pipeline: block awaiting
                    # attnT/oT/Wo after its dots+softmax were issued
                    pend = None      # (t, dots_ps, qT)

                    xb_pre = {}

                    def emit_dots(t, qT):
                        tb = t % 4
                        # prefetch the residual rows for this block's Wo step
                        r = b * d.n_blk + t
                        xb = xbin.tile([128, d.D], F32, tag="xres")
                        nc.sync.dma_start(xb[:], src.ap()[r * 128:(r + 1) * 128, :])
                        xb_pre[t] = xb
                        dots_ps = [pmm.tile([128, 512], F32, tag="mm",
                                            name=f"dots{j}") for j in range(2)]
                        for c in range(nD):
                            nc.tensor.matmul(
                                dots_ps[c // 4][:, (c % 4) * 128:(c % 4 + 1) * 128],
                                qT[:, c, tb * 128:(tb + 1) * 128],
                                bdT[:])
                        # softmax (bf16 exp, one broadcast mul)
                        expt = work.tile([128, d.H, d.K], BF16, tag="expt")
                        for j in range(2):
                            nc.scalar.activation(
                                expt[:, j * 8:(j + 1) * 8, :].rearrange("p a f -> p (a f)"),
                                dots_ps[j][:], AF.Exp, scale=float(d.DH) ** -0.5)
                        se = small.tile([128, d.H], F32, tag="se")
                        nc.vector.reduce_sum(se[:], expt[:], axis=AX.X)
                        rse = small.tile([128, d.H], F32, tag="rse")
                        nc.vector.reciprocal(rse[:], se[:])
                        attn = work.tile([128, d.H, d.K], BF16, tag="attn")
                        nc.vector.tensor_tensor(
                            attn[:], expt[:],
                            rse[:].unsqueeze(2).broadcast_to([128, d.H, d.K]),
                            ALU.mult)
                        if dbg and l == 0 and b == 0 and t == 0:
                            nc.sync.dma_start(
                                Ad.ap(), attn[:].rearrange("p a f -> p (a f)"))
                        return attn

                    def emit_tail(t, attn):
                        r = b * d.n_blk + t
                        attnT = work.tile([128, nD, 128], BF16, tag="attnT")
                        transpose_into(attn[:].rearrange("p a f -> p (a f)"),
                                       attnT, 0, 0, ptr)
                        oT_ps = [pmm.tile([128, 512], F32, tag="mm",
                                          name=f"oTps{j}") for j in range(2)]
                        for c in range(nD):
                            nc.tensor.matmul(
                                oT_ps[c // 4][:, (c % 4) * 128:(c % 4 + 1) * 128],
                                bdv[:],
                                attnT[:, c, :])
                        oT = work.tile([128, nD, 128], BF16, tag="oT")
                        for j in range(2):
                            nc.scalar.copy(
                                oT[:, j * 4:(j + 1) * 4, :],
                                oT_ps[j][:].rearrange("p (a f) -> p a f", a=4))
                        if dbg and l == 0 and b == 0 and t == 0:
                            nc.sync.dma_start(
                                Od.ap().rearrange("(a p) f -> p a f", p=128),
                                oT[:])
                        return r, oT

                    def emit_wo(r, oT):
                        xb = xb_pre.pop(r - b * d.n_blk)
                        xp = outp.tile([128, d.D], F32, tag="xp")
                        for ncol in range(2):
                            y_ps = pmm.tile([128, 512], F32, tag="mm")
                            for dc in range(nD):
                                nc.tensor.matmul(
                                    y_ps[:], oT[:, dc, :],
                                    woS[:, dc, ncol * 512:(ncol + 1) * 512],
                                    start=(dc == 0),
                                    stop=(dc == nD - 1 and ("bo", l) not in bias_sb))
                            if ("bo", l) in bias_sb:
                                nc.tensor.matmul(
                                    y_ps[:], ones_bf[:, 0:128],
                                    bias_sb[("bo", l)][:, ncol * 512:(ncol + 1) * 512],
                                    start=False, stop=True)
                            nc.vector.scalar_tensor_tensor(
                                xp[:, ncol * 512:(ncol + 1) * 512], y_ps[:], 1.0,
                                xb[:, ncol * 512:(ncol + 1) * 512], ALU.mult, ALU.add)
                        bn_mv(xp[:], s_mid[l][:, r, :])
                        nc.sync.dma_start(Xp.ap()[r * 128:(r + 1) * 128, :], xp[:])

                    for c4 in range(d.n_chunk):
                        tok0 = c4 * 512
                        qT = work.tile([128, nD, 512], BF16, tag="qT", bufs=1)
                        for ncol in range(nD):
                            q_ps = pmm.tile([128, 512], F32, tag="mm")
                            for dc in range(nD):
                                nc.tensor.matmul(
                                    q_ps[:], wqS[:, dc, ncol * 128:(ncol + 1) * 128],
                                    hT[:, dc, tok0:tok0 + 512],
                                    start=(dc == 0),
                                    stop=(dc == nD - 1 and ("cq", l) not in bias_sb))
                            if ("cq", l) in bias_sb:
                                nc.tensor.matmul(
                                    q_ps[:], bias_sb[("cq", l)][:, ncol * 128:(ncol + 1) * 128],
                                    ones_bf[:], start=False, stop=True)
                            nc.scalar.copy(qT[:, ncol, :], q_ps[:])
                            if dbg and l == 0 and b == 0 and c4 == 0:
                                nc.sync.dma_start(
                                    Qd.ap()[ncol * 128:(ncol + 1) * 128, :],
                                    qT[:, ncol, :])

                        for tb in range(4):
                            t = c4 * 4 + tb
                            if pend is not None:
                                pr, poT = emit_tail(*pend)
                            attn = emit_dots(t, qT)
                            if nxt is not None:
                                pass_a_block(nxt, t)
                            if pend is not None:
                                emit_wo(pr, poT)
                            pend = (t, attn)
                    # flush pipeline for this sequence
                    pr, poT = emit_tail(*pend)
                    emit_wo(pr, poT)
                    if nxt is not None:
                        bd_build(nxt)

            # =================== FF phase ==============================
            with tc.tile_pool(name=f"wff{l}", bufs=1) as wff, \
                 tc.tile_pool(name=f"phf{l}", bufs=1) as htp2, \
                 tc.tile_pool(name=f"h2p{l}", bufs=2) as h2p, \
                 tc.tile_pool(name=f"xf{l}", bufs=2) as xinf, \
                 tc.tile_pool(name=f"xrf{l}", bufs=1) as xres_p, \
                 tc.tile_pool(name=f"hf{l}", bufs=1) as hbuf2, \
                 tc.tile_pool(name=f"of{l}", bufs=1) as outp2, \
                 tc.tile_pool(name=f"pmf{l}", bufs=(4 if last else 6), space="PSUM") as pmf, \
                 tc.tile_pool(name=f"ptf{l}", bufs=2, space="PSUM") as ptf, \
                 ExitStack() as fctx:
                if last:
                    pzp = fctx.enter_context(
                        tc.tile_pool(name=f"pzp{l}", bufs=1, space="PSUM"))
                    peps = fctx.enter_context(
                        tc.tile_pool(name=f"peps{l}", bufs=1, space="PSUM"))
                    mus = [htp2.tile([128, nF], F32, tag=f"mu{b}", name=f"mu{b}")
                           for b in range(d.B_loc)]
                    for b in range(d.B_loc):
                        nc.vector.memset(mus[b][:], 0.0)
                    eps_cur = [None]

                rstd2 = stats.tile([128, nblk], F32, tag="rstd2", name=f"rstd2_{l}")
                nmr2 = stats.tile([128, nblk], F32, tag="nmr2", name=f"nmr2_{l}")
                convert_stats(s_mid[l], rstd2, nmr2)

                def prep_chunk(cg):
                    h2T = h2p.tile([128, nD, 512], BF16, tag="h2T",
                                   name=f"h2T{l}_{cg}")
                    bseq = cg // d.n_chunk
                    for tb in range(4):
                        r = cg * 4 + tb
                        xt = xinf.tile([128, d.D], F32, tag="xt")
                        nc.sync.dma_start(xt[:], Xp.ap()[r * 128:(r + 1) * 128, :])
                        h2 = hbuf2.tile([128, d.D], BF16, tag="h")
                        nc.gpsimd.tensor_scalar(h2[:], xt[:], rstd2[:, r:r + 1],
                                                nmr2[:, r:r + 1], ALU.mult, ALU.add)
                        if last:
                            # token-sum accumulation for the final mean-pool
                            t = r - bseq * d.n_blk
                            if t == 0:
                                eps_cur[0] = peps.tile([64, 512], F32, tag="eps",
                                                       name=f"eps{bseq}")
                            xtb = hbuf2.tile([128, d.D], BF16, tag="xtb")
                            nc.gpsimd.tensor_copy(xtb[:], xt[:])
                            for j in range(2):
                                nc.tensor.matmul(
                                    eps_cur[0][32 * j:32 * j + 1, :], ones_bc[:],
                                    xtb[:, j * 512:(j + 1) * 512],
                                    start=(t == 0), stop=(t == d.n_blk - 1))
                        transpose_into(h2[:], h2T, 0, tb, ptf)
                    if last and cg % d.n_chunk == d.n_chunk - 1:
                        esb = small.tile([1, 2, 512], F32, tag="ezsb", bufs=1)
                        for j in range(2):
                            nc.vector.tensor_copy(esb[:, j, :],
                                                  eps_cur[0][32 * j:32 * j + 1, :])
                        nc.sync.dma_start(Es.ap()[bseq:bseq + 1, :, :], esb[:])
                    return h2T

                h2T_next = prep_chunk(0)
                w1src = w1_d[l].ap().rearrange("(a p) n -> p a n", p=128)
                w1gs = []
                for g in range(4):
                    w1t = wff.tile([128, nD, d.FF // 4], BF16, tag=f"w1g{g}",
                                   name=f"w1_{l}_{g}")
                    nc.sync.dma_start(w1t[:], w1src[:, :, g * 1024:(g + 1) * 1024])
                    w1gs.append(w1t)
                w2src = w2_d[l].ap().rearrange("(a p) n -> p a n", p=128)
                w2gs = []
                for g in range(4):
                    w2t = wff.tile([128, nF // 4, d.D], BF16, tag=f"w2g{g}",
                                   name=f"w2_{l}_{g}")
                    nc.sync.dma_start(w2t[:], w2src[:, g * 8:(g + 1) * 8, :])
                    w2gs.append(w2t)
                bias_sb = load_bias_rows(wff, l, ("bu", "bz"))

                for cg in range(d.R // 512):
                    h2T = h2T_next
                    if cg + 1 < d.R // 512:
                        h2T_next = prep_chunk(cg + 1)
                    if last:
                        # pool trick: only token-sums of gelu(u) are needed;
                        # per-fc small tiles + immediate reduce (no big uT)
                        b = cg // d.n_chunk
                        red = small.tile([128, nF], F32, tag="red")
                        for fc in range(nF):
                            u_ps = pmf.tile([128, 512], F32, tag="mm")
                            for dc in range(nD):
                                nc.tensor.matmul(
                                    u_ps[:], w1gs[fc // 8][:, dc, (fc % 8) * 128:(fc % 8 + 1) * 128],
                                    h2T[:, dc, :], start=(dc == 0),
                                    stop=(dc == nD - 1 and ("bu", l) not in bias_sb))
                            if ("bu", l) in bias_sb:
                                nc.tensor.matmul(
                                    u_ps[:], bias_sb[("bu", l)][:, fc * 128:(fc + 1) * 128],
                                    ones_bf[:], start=False, stop=True)
                            us = hbuf2.tile([128, 512], BF16, tag="us", bufs=4)
                            nc.scalar.activation(us[:], u_ps[:], AF.Gelu)
                            nc.vector.reduce_sum(red[:, fc:fc + 1], us[:], axis=AX.X)
                        nc.vector.tensor_add(mus[b][:], mus[b][:], red[:])
                        if cg % d.n_chunk == d.n_chunk - 1:
                            # z-pool for this sequence: (mean_t gelu) @ W2
                            mu_bf = small.tile([128, nF], BF16, tag="mubf")
                            nc.scalar.mul(mu_bf[:], mus[b][:], 1.0 / d.N)
                            z_ps = pzp.tile([64, 512], F32, tag="zps",
                                            name=f"zps{b}")
                            for j in range(2):
                                for fc in range(nF):
                                    nc.tensor.matmul(
                                        z_ps[32 * j:32 * j + 1, :],
                                        mu_bf[:, fc:fc + 1],
                                        w2gs[fc // 8][:, fc % 8, j * 512:(j + 1) * 512],
                                        start=(fc == 0), stop=(fc == nF - 1))
                            zsb = small.tile([1, 2, 512], F32, tag="ezsb", bufs=1)
                            for j in range(2):
                                nc.vector.tensor_copy(
                                    zsb[:, j, :], z_ps[32 * j:32 * j + 1, :])
                            nc.sync.dma_start(Zs.ap()[b:b + 1, :, :], zsb[:])
                        continue
                    uT = htp2.tile([128, nF, 512], BF16, tag="uT")
                    for fc in range(nF):
                        u_ps = pmf.tile([128, 512], F32, tag="mm")
                        for dc in range(nD):
                            nc.tensor.matmul(
                                u_ps[:], w1gs[fc // 8][:, dc, (fc % 8) * 128:(fc % 8 + 1) * 128],
                                h2T[:, dc, :], start=(dc == 0),
                                stop=(dc == nD - 1 and ("bu", l) not in bias_sb))
                        if ("bu", l) in bias_sb:
                            nc.tensor.matmul(
                                u_ps[:], bias_sb[("bu", l)][:, fc * 128:(fc + 1) * 128],
                                ones_bf[:], start=False, stop=True)
                        nc.scalar.activation(uT[:, fc, :], u_ps[:], AF.Gelu)
                    for tb in range(4):
                        r = cg * 4 + tb
                        xres = xres_p.tile([128, d.D], F32, tag="xres")
                        nc.sync.dma_start(xres[:], Xp.ap()[r * 128:(r + 1) * 128, :])
                        xo = outp2.tile([128, d.D], F32, tag="xo")
                        for ncol in range(2):
                            z_ps = pmf.tile([128, 512], F32, tag="mm")
                            for fc in range(nF):
                                nc.tensor.matmul(
                                    z_ps[:], uT[:, fc, tb * 128:(tb + 1) * 128],
                                    w2gs[fc // 8][:, fc % 8, ncol * 512:(ncol + 1) * 512],
                                    start=(fc == 0),
                                    stop=(fc == nF - 1 and ("bz", l) not in bias_sb))
                            if ("bz", l) in bias_sb:
                                nc.tensor.matmul(
                                    z_ps[:], ones_bf[:, 0:128],
                                    bias_sb[("bz", l)][:, ncol * 512:(ncol + 1) * 512],
                                    start=False, stop=True)
                            nc.vector.scalar_tensor_tensor(
                                xo[:, ncol * 512:(ncol + 1) * 512], z_ps[:], 1.0,
                                xres[:, ncol * 512:(ncol + 1) * 512],
                                ALU.mult, ALU.add)
                        bn_mv(xo[:], s_in[l + 1][:, r, :])
                        nc.sync.dma_start(Xr.ap()[r * 128:(r + 1) * 128, :], xo[:])

        # ---------------- final: emb = sum_t Xp3 / N + z_pool; layernorm ---
        fin = ctx.enter_context(tc.tile_pool(name="fin", bufs=2))
        gt = fin.tile([1, d.D], F32, tag="lnfg", bufs=1)
        nc.sync.dma_start(gt[:], lnfg.ap())
        bt = fin.tile([1, d.D], F32, tag="lnfb", bufs=1)
        nc.sync.dma_start(bt[:], lnfb.ap())
        for b in range(d.B_loc):
            et = fin.tile([1, 2, 512], F32, tag="et")
            nc.sync.dma_start(et[:], Es.ap()[b:b + 1, :, :])
            zt = fin.tile([1, 2, 512], F32, tag="zt")
            nc.sync.dma_start(zt[:], Zs.ap()[b:b + 1, :, :])
            emb = fin.tile([1, d.D], F32, tag="emb")
            for j in range(2):
                nc.vector.scalar_tensor_tensor(
                    emb[:, j * 512:(j + 1) * 512], et[:, j, :], 1.0 / d.N,
                    zt[:, j, :], ALU.mult, ALU.add)
            bns = small.tile([1, 2, 6], F32, tag="fbns")
            for g in range(2):
                nc.vector.bn_stats(bns[:, g, :], emb[:, g * 512:(g + 1) * 512])
            mv = small.tile([1, 2], F32, tag="fmv")
            nc.vector.bn_aggr(mv[:], bns[:])
            lnv = small.tile([1, 1], F32, tag="flnv")
            nc.scalar.activation(lnv[:], mv[:, 1:2], AF.Ln, bias=eps_t[:1, :])
            rstd = small.tile([1, 1], F32, tag="frstd")
            nc.scalar.activation(rstd[:], lnv[:], AF.Exp, scale=-0.5)
            nmr = small.tile([1, 1], F32, tag="fnmr")
            nc.vector.scalar_tensor_tensor(nmr[:], mv[:, 0:1], -1.0, rstd[:],
                                           ALU.mult, ALU.mult)
            nrm = fin.tile([1, d.D], F32, tag="nrm")
            nc.vector.tensor_scalar(nrm[:], emb[:], rstd[:], nmr[:],
                                    ALU.mult, ALU.add)
            ot = fin.tile([1, d.D], F32, tag="ot")
            nc.vector.tensor_mul(ot[:], nrm[:], gt[:])
            nc.vector.tensor_add(ot[:], ot[:], bt[:])
            nc.sync.dma_start(out.ap()[b:b + 1, :], ot[:])

    nc.compile()
    return nc


_CACHE = {}


def _to_bf16(a):
    return np.asarray(a, dtype=np.float32).astype(ml_dtypes.bfloat16)


def prepare_inputs(dims: Dims, n_cores, token_ids, token_emb, pos_emb, ln1_g, ln1_b,
                   Wq, Wk, Pk, Wo, bo, ln2_g, ln2_b, W1, b1, W2, b2, lnf_g, lnf_b):
    d = dims
    token_ids = np.asarray(token_ids)
    token_emb = np.asarray(token_emb, dtype=np.float32)
    pos_emb = np.asarray(pos_emb, dtype=np.float32)

    x_all = token_emb[token_ids[:, :d.N]] + pos_emb[None, :d.N, :]  # [B, N, D]
    B = token_ids.shape[0]
    assert B == n_cores * d.B_loc

    biases = {}
    shared = {}
    for l in range(d.L):
        g1 = np.asarray(ln1_g[l], np.float32)
        b1l = np.asarray(ln1_b[l], np.float32)
        g2 = np.asarray(ln2_g[l], np.float32)
        Wql = np.asarray(Wq[l], np.float32)
        Wkl = np.asarray(Wk[l], np.float32)
        W1l = np.asarray(W1[l], np.float32)
        shared[f"wq{l}"] = _to_bf16(g1[:, None] * Wql)
        shared[f"wk{l}"] = _to_bf16(g1[:, None] * Wkl)
        shared[f"pk{l}"] = _to_bf16(np.asarray(Pk[l])[:d.N])
        shared[f"wo{l}"] = _to_bf16(Wo[l])
        shared[f"w1{l}"] = _to_bf16(g2[:, None] * W1l)
        shared[f"w2{l}"] = _to_bf16(W2[l])

        def nz(v):
            v = np.asarray(v, np.float32)
            return v if np.any(v != 0) else None

        cq = nz(b1l @ Wql)
        bul = nz(np.asarray(ln2_b[l], np.float32) @ W1l + np.asarray(b1[l], np.float32))
        bol = nz(bo[l])
        bzl = nz(b2[l])
        biases[("cq", l)] = _to_bf16(cq) if cq is not None else None
        biases[("bu", l)] = _to_bf16(bul) if bul is not None else None
        biases[("bo", l)] = _to_bf16(bol) if bol is not None else None
        biases[("bz", l)] = _to_bf16(bzl) if bzl is not None else None

    lnf_g_rep = np.asarray(lnf_g, np.float32).reshape(1, d.D).copy()
    lnf_b_rep = np.asarray(lnf_b, np.float32).reshape(1, d.D).copy()

    in_maps = []
    for c in range(n_cores):
        m = dict(shared)
        xc = np.ascontiguousarray(
            x_all[c * d.B_loc:(c + 1) * d.B_loc].reshape(d.R, d.D), dtype=np.float32)
        m["x0"] = xc
        # layer-0 LN stats, laid out [128, n_blocks, 2]
        mean = xc.mean(axis=1).astype(np.float32)       # [R]
        var = xc.var(axis=1).astype(np.float32)         # [R]
        s = np.stack([mean, var], axis=-1).reshape(d.R // 128, 128, 2)
        m["s1_0"] = np.ascontiguousarray(s.transpose(1, 0, 2))
        m["lnfg"] = lnf_g_rep
        m["lnfb"] = lnf_b_rep
        for key, v in biases.items():
            if v is not None:
                m[f"{key[0]}{key[1]}"] = v
        in_maps.append(m)
    return in_maps, biases


def run(dims: Dims, n_cores, inputs, trace=False, tmpdir=None):
    in_maps, biases = prepare_inputs(dims, n_cores, **inputs)
    ck = (dims, n_cores, tuple(sorted(k for k, v in biases.items() if v is not None)))
    if ck not in _CACHE:
        _CACHE[ck] = build(dims, n_cores, biases)
    nc = _CACHE[ck]
    res = run_bass_kernel_spmd(nc, in_maps, list(range(n_cores)), trace=trace,
                               tmpdir=tmpdir)
    outs = np.concatenate([res.results[i]["out"] for i in range(n_cores)], axis=0)
    return outs, res


def kernel(**inputs) -> np.ndarray:
    out, _ = run(Dims(), 8, inputs)
    return out.astype(np.float32)


# revision 25
# speedup vs baseline: 1.0984x; 1.0984x over previous
"""Linformer text encoder on 8 TRN2 NeuronCores.

Sharding: pure data-parallel over batch (32 seqs -> 4 per core), weights
replicated, no collectives. Host does the embedding gather and folds LN
gamma into the following projection weights (exact math), plus computes
layer-0 LN stats. Device does everything else in bf16 matmuls with f32
accumulation.

Engine plan (v2):
  - ScalarE: softmax Exp + psum->sbuf copies (COPY needs no act table) +
    per-phase batched LN stat conversion (Ln/Exp loaded once per phase).
  - GpSimd: LN affine applies (sbuf f32 -> sbuf bf16).
  - DVE: bn_stats, softmax reduce/recip/broadcast-mul, residual adds,
    transpose psum->sbuf copies.
  - PE: matmuls + transposes, software-pipelined so softmax latency is
    hidden (keeps HAM at full clock).
  - Final token-mean folded into layer 3 (ones-matmul accumulation in
    attention pass + pooled-W2 trick in FF), so no final re-read of Xp.

Self-contained: hardcodes all shapes from the problem spec.
"""

import sys

sys.path.insert(0, "/opt/trn_rl_repo")

from contextlib import ExitStack
from dataclasses import dataclass

import ml_dtypes
import numpy as np

import concourse.bass as bass
import concourse.tile as tile
from concourse import bacc, mybir
from concourse.bass_utils import run_bass_kernel_spmd
from concourse.masks import make_identity

F32 = mybir.dt.float32
BF16 = mybir.dt.bfloat16
AF = mybir.ActivationFunctionType
ALU = mybir.AluOpType
AX = mybir.AxisListType

EPS = 1e-5


@dataclass(frozen=True)
class Dims:
    B_loc: int = 4      # sequences per core
    N: int = 2048       # tokens per sequence
    D: int = 1024
    H: int = 16
    DH: int = 64
    K: int = 64
    FF: int = 4096
    L: int = 4

    @property
    def R(self):
        return self.B_loc * self.N

    @property
    def n_blk(self):        # 128-token blocks per sequence
        return self.N // 128

    @property
    def n_chunk(self):      # 512-token chunks per sequence
        return self.N // 512


def build(dims: Dims, n_cores: int, biases, dbg=False):
    """Emit the full per-core program. `biases` is a dict of host numpy
    vectors (cq, bu, bo, bz per layer) or None entries when zero."""
    d = dims
    nc = bacc.Bacc("TRN2", target_bir_lowering=False, debug=False,
                   num_devices=n_cores, enable_asserts=False)

    x0 = nc.dram_tensor("x0", [d.R, d.D], F32, kind="ExternalInput")
    s1_0 = nc.dram_tensor("s1_0", [128, d.R // 128, 2], F32, kind="ExternalInput")
    wq_d = [nc.dram_tensor(f"wq{l}", [d.D, d.D], BF16, kind="ExternalInput") for l in range(d.L)]
    wk_d = [nc.dram_tensor(f"wk{l}", [d.D, d.DH], BF16, kind="ExternalInput") for l in range(d.L)]
    pk_d = [nc.dram_tensor(f"pk{l}", [d.N, d.K], BF16, kind="ExternalInput") for l in range(d.L)]
    wo_d = [nc.dram_tensor(f"wo{l}", [d.D, d.D], BF16, kind="ExternalInput") for l in range(d.L)]
    w1_d = [nc.dram_tensor(f"w1{l}", [d.D, d.FF], BF16, kind="ExternalInput") for l in range(d.L)]
    w2_d = [nc.dram_tensor(f"w2{l}", [d.FF, d.D], BF16, kind="ExternalInput") for l in range(d.L)]
    lnfg = nc.dram_tensor("lnfg", [1, d.D], F32, kind="ExternalInput")
    lnfb = nc.dram_tensor("lnfb", [1, d.D], F32, kind="ExternalInput")
    out = nc.dram_tensor("out", [d.B_loc, d.D], F32, kind="ExternalOutput")

    bias_d = {}
    for l in range(d.L):
        for nm in ("cq", "bu", "bo", "bz"):
            if biases and biases.get((nm, l)) is not None:
                arr = biases[(nm, l)]
                bias_d[(nm, l)] = nc.dram_tensor(
                    f"{nm}{l}", list(arr.shape), BF16, kind="ExternalInput")

    ikind = "ExternalOutput" if dbg else "Internal"
    Xp = nc.dram_tensor("Xp", [d.R, d.D], F32, kind=ikind)   # post-attention residual
    Xr = nc.dram_tensor("Xr", [d.R, d.D], F32, kind=ikind)   # post-FF residual
    if dbg:
        Hd = nc.dram_tensor("Hd", [d.R, d.D], BF16, kind="ExternalOutput")
        KVd = nc.dram_tensor("KVd", [2 * d.DH, d.K], BF16, kind="ExternalOutput")
        Qd = nc.dram_tensor("Qd", [d.D, 512], BF16, kind="ExternalOutput")
        Ad = nc.dram_tensor("Ad", [128, d.H * d.K], BF16, kind="ExternalOutput")
        Od = nc.dram_tensor("Od", [d.D, 128], BF16, kind="ExternalOutput")
    Es = nc.dram_tensor("Es", [d.B_loc, 2, 512], F32)  # l3 token-sums of Xp
    Zs = nc.dram_tensor("Zs", [d.B_loc, 2, 512], F32)  # l3 pooled FF delta

    nD = d.D // 128       # 8 feature chunks
    nF = d.FF // 128      # 32 ff chunks
    nblk = d.R // 128     # 64 token blocks per core

    with ExitStack() as ctx:
        tc = ctx.enter_context(tile.TileContext(nc))
        const = ctx.enter_context(tc.tile_pool(name="const", bufs=1))
        small = ctx.enter_context(tc.tile_pool(name="small", bufs=4))
        stats = ctx.enter_context(tc.tile_pool(name="stats", bufs=1))

        idt = const.tile([128, 128], BF16)
        make_identity(nc, idt[:])
        ones_bf = const.tile([1, 512], BF16)
        nc.vector.memset(ones_bf[:], 1.0)
        ones_bc = const.tile([128, 1], BF16)
        nc.vector.memset(ones_bc[:], 1.0)
        eps_t = const.tile([128, 1], F32)
        nc.vector.memset(eps_t[:], EPS)
        ilo = const.tile([64, 128], BF16)
        nc.vector.memset(ilo[:], 0.0)
        make_identity(nc, ilo[:, 0:64])
        ihi = const.tile([64, 128], BF16)
        nc.vector.memset(ihi[:], 0.0)
        make_identity(nc, ihi[:, 64:128])

        # persistent stat tiles per layer: s_in (LN1 stats of layer input),
        # s_mid (LN2 stats of post-attn residual)
        s_in = [stats.tile([128, nblk, 2], F32, name=f"sin{l}") for l in range(d.L)]
        s_mid = [stats.tile([128, nblk, 2], F32, name=f"smid{l}") for l in range(d.L)]
        nc.sync.dma_start(s_in[0][:], s1_0.ap())

        def bn_mv(xt_ap, out_mv):
            """bn stats of [128, D] f32 -> out_mv [128, 2] = (mean, var)."""
            ngrp = xt_ap.shape[1] // 512
            bns = small.tile([128, ngrp, 6], F32, tag="bns")
            for g in range(ngrp):
                nc.vector.bn_stats(bns[:, g, :], xt_ap[:, g * 512:(g + 1) * 512])
            nc.vector.bn_aggr(out_mv, bns[:])

        def convert_stats(src, rstd, nmr):
            """Batched (mean,var)[128, nblk, 2] -> rstd/nmr [128, nblk].
            Two act-table loads total (Ln, Exp)."""
            lnv = small.tile([128, nblk], F32, tag="lnv", bufs=1)
            nc.scalar.activation(lnv[:], src[:, :, 1], AF.Ln, bias=eps_t[:])
            nc.scalar.activation(rstd[:], lnv[:], AF.Exp, scale=-0.5)
            nc.vector.scalar_tensor_tensor(nmr[:], src[:, :, 0], -1.0, rstd[:],
                                           ALU.mult, ALU.mult)

        def transpose_into(h_ap, dst_tile, dst_chunk0, tcol, ptr):
            """Transpose h_ap [128, nch*128]: chunk c -> dst[:, dst_chunk0+c,
            tcol*128:+128]. PE transposes + DVE copies."""
            nch = h_ap.shape[1] // 128
            for c0 in range(0, nch, 4):
                cn = min(4, nch - c0)
                pt = ptr.tile([128, 512], BF16, tag="pt")
                for c in range(cn):
                    nc.tensor.transpose(
                        pt[:, c * 128:(c + 1) * 128],
                        h_ap[:, (c0 + c) * 128:(c0 + c + 1) * 128], idt[:])
                nc.vector.tensor_copy(
                    dst_tile[:, dst_chunk0 + c0:dst_chunk0 + c0 + cn,
                             tcol * 128:(tcol + 1) * 128],
                    pt[:, :cn * 128].rearrange("p (a f) -> p a f", a=cn))

        def load_bias_rows(pool, l, names):
            out = {}
            for nm in names:
                if (nm, l) in bias_d:
                    dram = bias_d[(nm, l)]
                    t = pool.tile([1, dram.shape[0]], BF16, tag=f"b{nm}",
                                  name=f"b{nm}{l}")
                    nc.sync.dma_start(t[:], dram.ap()[None, :])
                    out[(nm, l)] = t
            return out

        for l in range(d.L):
            src = x0 if l == 0 else Xr
            last = l == d.L - 1

            # =================== attention phase =======================
            with tc.tile_pool(name=f"wat{l}", bufs=1) as wat, \
                 tc.tile_pool(name=f"pha{l}", bufs=2) as htp, \
                 tc.tile_pool(name=f"wka{l}", bufs=2) as work, \
                 tc.tile_pool(name=f"xa{l}", bufs=3) as xin, \
                 tc.tile_pool(name=f"xba{l}", bufs=3) as xbin, \
                 tc.tile_pool(name=f"ha{l}", bufs=2) as hbuf, \
                 tc.tile_pool(name=f"oa{l}", bufs=3) as outp, \
                 tc.tile_pool(name=f"pmm{l}", bufs=5, space="PSUM") as pmm, \
                 tc.tile_pool(name=f"ptr{l}", bufs=2, space="PSUM") as ptr, \
                 tc.tile_pool(name=f"pkv{l}", bufs=1, space="PSUM") as pkv:
                rstd1 = stats.tile([128, nblk], F32, tag="rstd1", name=f"rstd1_{l}")
                nmr1 = stats.tile([128, nblk], F32, tag="nmr1", name=f"nmr1_{l}")
                convert_stats(s_in[l], rstd1, nmr1)

                wqS = wat.tile([128, nD, d.D], BF16, tag="wq")
                nc.sync.dma_start(wqS[:], wq_d[l].ap().rearrange("(a p) n -> p a n", p=128))
                wkS = wat.tile([128, nD, d.DH], BF16, tag="wk")
                nc.sync.dma_start(wkS[:], wk_d[l].ap().rearrange("(a p) n -> p a n", p=128))
                pkS = wat.tile([128, d.n_blk, d.K], BF16, tag="pk")
                nc.sync.dma_start(pkS[:], pk_d[l].ap().rearrange("(a p) k -> p a k", p=128))
                woS = wat.tile([128, nD, d.D], BF16, tag="wo")
                nc.sync.dma_start(woS[:], wo_d[l].ap().rearrange("(a p) n -> p a n", p=128))
                bias_sb = load_bias_rows(wat, l, ("cq", "bo"))

                hTs = {}
                cur_kv = {}
                cur_bd = {}
                xt_pre = {}

                def prefetch_xt(r):
                    t = xin.tile([128, d.D], F32, tag="xt")
                    nc.sync.dma_start(t[:], src.ap()[r * 128:(r + 1) * 128, :])
                    xt_pre[r] = t

                def pass_a_block(b, t):
                    r = b * d.n_blk + t
                    if t == 0:
                        hTs[b] = htp.tile([128, nD, d.N], BF16, tag="hT",
                                          name=f"hT{l}_{b}")
                        kvp = pkv.tile([128, 64], F32, tag="kvp", name=f"kvp{l}_{b}")
                        cur_kv[b] = (kvp[0:64, :], kvp[64:128, :])
                    hT = hTs[b]
                    kvT_ps, kv_ps = cur_kv[b]
                    if r not in xt_pre:
                        prefetch_xt(r)
                    xt = xt_pre.pop(r)
                    h = hbuf.tile([128, d.D], BF16, tag="h")
                    nc.gpsimd.tensor_scalar(h[:], xt[:], rstd1[:, r:r + 1],
                                            nmr1[:, r:r + 1], ALU.mult, ALU.add)
                    if dbg and l == 0:
                        nc.sync.dma_start(Hd.ap()[r * 128:(r + 1) * 128, :], h[:])
                    transpose_into(h[:], hT, 0, t, ptr)
                    hk_ps = pmm.tile([128, d.DH], F32, tag="mm", name=f"hkps{l}_{r}")
                    for dc in range(nD):
                        nc.tensor.matmul(
                            hk_ps[:], hT[:, dc, t * 128:(t + 1) * 128],
                            wkS[:, dc, :], start=(dc == 0), stop=(dc == nD - 1))
                    hk = work.tile([128, d.DH], BF16, tag="hk")
                    nc.scalar.copy(hk[:], hk_ps[:])
                    nc.tensor.matmul(kvT_ps[:], hk[:], pkS[:, t, :],
                                     start=(t == 0), stop=(t == d.n_blk - 1))
                    nc.tensor.matmul(kv_ps[:], pkS[:, t, :], hk[:],
                                     start=(t == 0), stop=(t == d.n_blk - 1))
                    # prefetch next pass-A xt up to two blocks ahead
                    for dt in (1, 2):
                        if t + dt < d.n_blk and (r + dt) not in xt_pre:
                            prefetch_xt(r + dt)

                def bd_build(b):
                    kvT_ps, kv_ps = cur_kv.pop(b)
                    kvT = work.tile([64, 64], BF16, tag="kvT")
                    kv = work.tile([64, 64], BF16, tag="kv")
                    nc.scalar.copy(kvT[:], kvT_ps[:])
                    nc.scalar.copy(kv[:], kv_ps[:])
                    if dbg and l == 0 and b == 0:
                        nc.sync.dma_start(KVd.ap()[0:d.DH, :], kvT[:])
                        nc.sync.dma_start(KVd.ap()[d.DH:2 * d.DH, :], kv[:])
                    bdT_ps = pmm.tile([128, 128], F32, tag="mm", name="bdTps")
                    nc.tensor.matmul(bdT_ps[:, 0:64], ilo[:], kvT[:])
                    nc.tensor.matmul(bdT_ps[:, 64:128], ihi[:], kvT[:])
                    bdv_ps = pmm.tile([128, 128], F32, tag="mm", name="bdvps")
                    nc.tensor.matmul(bdv_ps[:, 0:64], ilo[:], kv[:])
                    nc.tensor.matmul(bdv_ps[:, 64:128], ihi[:], kv[:])
                    bdT = work.tile([128, 128], BF16, tag="bdT")
                    nc.scalar.copy(bdT[:], bdT_ps[:])
                    bdv = work.tile([128, 128], BF16, tag="bdv")
                    nc.scalar.copy(bdv[:], bdv_ps[:])
                    cur_bd[b] = (bdT, bdv)

                # ---- sequence-0 pass A, standalone ----
                prefetch_xt(0)
                for t in range(d.n_blk):
                    pass_a_block(0, t)
                bd_build(0)

                for b in range(d.B_loc):
                    hT = hTs.pop(b)
                    bdT, bdv = cur_bd.pop(b)
                    nxt = b + 1 if b + 1 < d.B_loc else None
                    if nxt is not None:
                        prefetch_xt(nxt * d.n_blk)

                    # state of the software pipeline: block awaiting
                    # attnT/oT/Wo after its dots+softmax were issued
                    pend = None      # (t, dots_ps, qT)

                    xb_pre = {}

                    def emit_dots(t, qT):
                        tb = t % 4
                        # prefetch the residual rows for this block's Wo step
                        r = b * d.n_blk + t
                        xb = xbin.tile([128, d.D], F32, tag="xres")
                        nc.sync.dma_start(xb[:], src.ap()[r * 128:(r + 1) * 128, :])
                        xb_pre[t] = xb
                        dots_ps = [pmm.tile([128, 512], F32, tag="mm",
                                            name=f"dots{j}") for j in range(2)]
                        for c in range(nD):
                            nc.tensor.matmul(
                                dots_ps[c // 4][:, (c % 4) * 128:(c % 4 + 1) * 128],
                                qT[:, c, tb * 128:(tb + 1) * 128],
                                bdT[:])
                        # softmax (bf16 exp, one broadcast mul)
                        expt = work.tile([128, d.H, d.K], BF16, tag="expt")
                        for j in range(2):
                            nc.scalar.activation(
                                expt[:, j * 8:(j + 1) * 8, :].rearrange("p a f -> p (a f)"),
                                dots_ps[j][:], AF.Exp, scale=float(d.DH) ** -0.5)
                        se = small.tile([128, d.H], F32, tag="se")
                        nc.vector.reduce_sum(se[:], expt[:], axis=AX.X)
                        rse = small.tile([128, d.H], F32, tag="rse")
                        nc.vector.reciprocal(rse[:], se[:])
                        attn = work.tile([128, d.H, d.K], BF16, tag="attn")
                        nc.vector.tensor_tensor(
                            attn[:], expt[:],
                            rse[:].unsqueeze(2).broadcast_to([128, d.H, d.K]),
                            ALU.mult)
                        if dbg and l == 0 and b == 0 and t == 0:
                            nc.sync.dma_start(
                                Ad.ap(), attn[:].rearrange("p a f -> p (a f)"))
                        return attn

                    def emit_tail(t, attn):
                        r = b * d.n_blk + t
                        attnT = work.tile([128, nD, 128], BF16, tag="attnT")
                        transpose_into(attn[:].rearrange("p a f -> p (a f)"),
                                       attnT, 0, 0, ptr)
                        oT_ps = [pmm.tile([128, 512], F32, tag="mm",
                                          name=f"oTps{j}") for j in range(2)]
                        for c in range(nD):
                            nc.tensor.matmul(
                                oT_ps[c // 4][:, (c % 4) * 128:(c % 4 + 1) * 128],
                                bdv[:],
                                attnT[:, c, :])
                        oT = work.tile([128, nD, 128], BF16, tag="oT")
                        for j in range(2):
                            nc.scalar.copy(
                                oT[:, j * 4:(j + 1) * 4, :],
                                oT_ps[j][:].rearrange("p (a f) -> p a f", a=4))
                        if dbg and l == 0 and b == 0 and t == 0:
                            nc.sync.dma_start(
                                Od.ap().rearrange("(a p) f -> p a f", p=128),
                                oT[:])
                        return r, oT

                    def emit_wo(r, oT):
                        xb = xb_pre.pop(r - b * d.n_blk)
                        xp = outp.tile([128, d.D], F32, tag="xp")
                        for ncol in range(2):
                            y_ps = pmm.tile([128, 512], F32, tag="mm")
                            for dc in range(nD):
                                nc.tensor.matmul(
                                    y_ps[:], oT[:, dc, :],
                                    woS[:, dc, ncol * 512:(ncol + 1) * 512],
                                    start=(dc == 0),
                                    stop=(dc == nD - 1 and ("bo", l) not in bias_sb))
                            if ("bo", l) in bias_sb:
                                nc.tensor.matmul(
                                    y_ps[:], ones_bf[:, 0:128],
                                    bias_sb[("bo", l)][:, ncol * 512:(ncol + 1) * 512],
                                    start=False, stop=True)
                            nc.vector.scalar_tensor_tensor(
                                xp[:, ncol * 512:(ncol + 1) * 512], y_ps[:], 1.0,
                                xb[:, ncol * 512:(ncol + 1) * 512], ALU.mult, ALU.add)
                        bn_mv(xp[:], s_mid[l][:, r, :])
                        nc.sync.dma_start(Xp.ap()[r * 128:(r + 1) * 128, :], xp[:])

                    for c4 in range(d.n_chunk):
                        tok0 = c4 * 512
                        qT = work.tile([128, nD, 512], BF16, tag="qT", bufs=1)
                        for ncol in range(nD):
                            q_ps = pmm.tile([128, 512], F32, tag="mm")
                            for dc in range(nD):
                                nc.tensor.matmul(
                                    q_ps[:], wqS[:, dc, ncol * 128:(ncol + 1) * 128],
                                    hT[:, dc, tok0:tok0 + 512],
                                    start=(dc == 0),
                                    stop=(dc == nD - 1 and ("cq", l) not in bias_sb))
                            if ("cq", l) in bias_sb:
                                nc.tensor.matmul(
                                    q_ps[:], bias_sb[("cq", l)][:, ncol * 128:(ncol + 1) * 128],
                                    ones_bf[:], start=False, stop=True)
                            nc.scalar.copy(qT[:, ncol, :], q_ps[:])
                            if dbg and l == 0 and b == 0 and c4 == 0:
                                nc.sync.dma_start(
                                    Qd.ap()[ncol * 128:(ncol + 1) * 128, :],
                                    qT[:, ncol, :])

                        for tb in range(4):
                            t = c4 * 4 + tb
                            if pend is not None:
                                pr, poT = emit_tail(*pend)
                            attn = emit_dots(t, qT)
                            if nxt is not None:
                                pass_a_block(nxt, t)
                            if pend is not None:
                                emit_wo(pr, poT)
                            pend = (t, attn)
                    # flush pipeline for this sequence
                    pr, poT = emit_tail(*pend)
                    emit_wo(pr, poT)
                    if nxt is not None:
                        bd_build(nxt)

            # =================== FF phase ==============================
            with tc.tile_pool(name=f"wff{l}", bufs=1) as wff, \
                 tc.tile_pool(name=f"phf{l}", bufs=1) as htp2, \
                 tc.tile_pool(name=f"h2p{l}", bufs=2) as h2p, \
                 tc.tile_pool(name=f"xf{l}", bufs=2) as xinf, \
                 tc.tile_pool(name=f"xrf{l}", bufs=1) as xres_p, \
                 tc.tile_pool(name=f"hf{l}", bufs=1) as hbuf2, \
                 tc.tile_pool(name=f"of{l}", bufs=1) as outp2, \
                 tc.tile_pool(name=f"pmf{l}", bufs=(4 if last else 6), space="PSUM") as pmf, \
                 tc.tile_pool(name=f"ptf{l}", bufs=2, space="PSUM") as ptf, \
                 ExitStack() as fctx:
                if last:
                    pzp = fctx.enter_context(
                        tc.tile_pool(name=f"pzp{l}", bufs=1, space="PSUM"))
                    peps = fctx.enter_context(
                        tc.tile_pool(name=f"peps{l}", bufs=1, space="PSUM"))
                    mus = [htp2.tile([128, nF], F32, tag=f"mu{b}", name=f"mu{b}")
                           for b in range(d.B_loc)]
                    for b in range(d.B_loc):
                        nc.vector.memset(mus[b][:], 0.0)
                    eps_cur = [None]

                rstd2 = stats.tile([128, nblk], F32, tag="rstd2", name=f"rstd2_{l}")
                nmr2 = stats.tile([128, nblk], F32, tag="nmr2", name=f"nmr2_{l}")
                convert_stats(s_mid[l], rstd2, nmr2)

                def prep_chunk(cg):
                    h2T = h2p.tile([128, nD, 512], BF16, tag="h2T",
                                   name=f"h2T{l}_{cg}")
                    bseq = cg // d.n_chunk
                    xts = []
                    for tb in range(4):
                        r = cg * 4 + tb
                        xt = xinf.tile([128, d.D], F32, tag="xt")
                        nc.sync.dma_start(xt[:], Xp.ap()[r * 128:(r + 1) * 128, :])
                        h2 = hbuf2.tile([128, d.D], BF16, tag="h")
                        nc.gpsimd.tensor_scalar(h2[:], xt[:], rstd2[:, r:r + 1],
                                                nmr2[:, r:r + 1], ALU.mult, ALU.add)
                        if last:
                            # token-sum accumulation for the final mean-pool
                            t = r - bseq * d.n_blk
                            if t == 0:
                                eps_cur[0] = peps.tile([64, 512], F32, tag="eps",
                                                       name=f"eps{bseq}")
                            xtb = hbuf2.tile([128, d.D], BF16, tag="xtb")
                            nc.gpsimd.tensor_copy(xtb[:], xt[:])
                            for j in range(2):
                                nc.tensor.matmul(
                                    eps_cur[0][32 * j:32 * j + 1, :], ones_bc[:],
                                    xtb[:, j * 512:(j + 1) * 512],
                                    start=(t == 0), stop=(t == d.n_blk - 1))
                        transpose_into(h2[:], h2T, 0, tb, ptf)
                    if last and cg % d.n_chunk == d.n_chunk - 1:
                        esb = small.tile([1, 2, 512], F32, tag="ezsb", bufs=1)
                        for j in range(2):
                            nc.vector.tensor_copy(esb[:, j, :],
                                                  eps_cur[0][32 * j:32 * j + 1, :])
                        nc.sync.dma_start(Es.ap()[bseq:bseq + 1, :, :], esb[:])
                    return h2T, xts

                h2T_next, xts_next = prep_chunk(0)
                w1src = w1_d[l].ap().rearrange("(a p) n -> p a n", p=128)
                w1gs = []
                for g in range(4):
                    w1t = wff.tile([128, nD, d.FF // 4], BF16, tag=f"w1g{g}",
                                   name=f"w1_{l}_{g}")
                    nc.sync.dma_start(w1t[:], w1src[:, :, g * 1024:(g + 1) * 1024])
                    w1gs.append(w1t)
                w2src = w2_d[l].ap().rearrange("(a p) n -> p a n", p=128)
                w2gs = []
                for g in range(4):
                    w2t = wff.tile([128, nF // 4, d.D], BF16, tag=f"w2g{g}",
                                   name=f"w2_{l}_{g}")
                    nc.sync.dma_start(w2t[:], w2src[:, g * 8:(g + 1) * 8, :])
                    w2gs.append(w2t)
                bias_sb = load_bias_rows(wff, l, ("bu", "bz"))

                for cg in range(d.R // 512):
                    h2T, xts_cur = h2T_next, xts_next
                    if cg + 1 < d.R // 512:
                        h2T_next, xts_next = prep_chunk(cg + 1)
                    if last:
                        # pool trick: only token-sums of gelu(u) are needed;
                        # per-fc small tiles + immediate reduce (no big uT)
                        b = cg // d.n_chunk
                        red = small.tile([128, nF], F32, tag="red")
                        for fc in range(nF):
                            u_ps = pmf.tile([128, 512], F32, tag="mm")
                            for dc in range(nD):
                                nc.tensor.matmul(
                                    u_ps[:], w1gs[fc // 8][:, dc, (fc % 8) * 128:(fc % 8 + 1) * 128],
                                    h2T[:, dc, :], start=(dc == 0),
                                    stop=(dc == nD - 1 and ("bu", l) not in bias_sb))
                            if ("bu", l) in bias_sb:
                                nc.tensor.matmul(
                                    u_ps[:], bias_sb[("bu", l)][:, fc * 128:(fc + 1) * 128],
                                    ones_bf[:], start=False, stop=True)
                            us = hbuf2.tile([128, 512], BF16, tag="us", bufs=4)
                            nc.scalar.activation(us[:], u_ps[:], AF.Gelu)
                            nc.vector.reduce_sum(red[:, fc:fc + 1], us[:], axis=AX.X)
                        nc.vector.tensor_add(mus[b][:], mus[b][:], red[:])
                        if cg % d.n_chunk == d.n_chunk - 1:
                            # z-pool for this sequence: (mean_t gelu) @ W2
                            mu_bf = small.tile([128, nF], BF16, tag="mubf")
                            nc.scalar.mul(mu_bf[:], mus[b][:], 1.0 / d.N)
                            z_ps = pzp.tile([64, 512], F32, tag="zps",
                                            name=f"zps{b}")
                            for j in range(2):
                                for fc in range(nF):
                                    nc.tensor.matmul(
                                        z_ps[32 * j:32 * j + 1, :],
                                        mu_bf[:, fc:fc + 1],
                                        w2gs[fc // 8][:, fc % 8, j * 512:(j + 1) * 512],
                                        start=(fc == 0), stop=(fc == nF - 1))
                            zsb = small.tile([1, 2, 512], F32, tag="ezsb", bufs=1)
                            for j in range(2):
                                nc.vector.tensor_copy(
                                    zsb[:, j, :], z_ps[32 * j:32 * j + 1, :])
                            nc.sync.dma_start(Zs.ap()[b:b + 1, :, :], zsb[:])
                        continue
                    uT = htp2.tile([128, nF, 512], BF16, tag="uT")
                    for fc in range(nF):
                        u_ps = pmf.tile([128, 512], F32, tag="mm")
                        for dc in range(nD):
                            nc.tensor.matmul(
                                u_ps[:], w1gs[fc // 8][:, dc, (fc % 8) * 128:(fc % 8 + 1) * 128],
                                h2T[:, dc, :], start=(dc == 0),
                                stop=(dc == nD - 1 and ("bu", l) not in bias_sb))
                        if ("bu", l) in bias_sb:
                            nc.tensor.matmul(
                                u_ps[:], bias_sb[("bu", l)][:, fc * 128:(fc + 1) * 128],
                                ones_bf[:], start=False, stop=True)
                        nc.scalar.activation(uT[:, fc, :], u_ps[:], AF.Gelu)
                    for tb in range(4):
                        r = cg * 4 + tb
                        xres = xres_p.tile([128, d.D], F32, tag="xres")
                        nc.sync.dma_start(xres[:], Xp.ap()[r * 128:(r + 1) * 128, :])
                        xo = outp2.tile([128, d.D], F32, tag="xo")
                        for ncol in range(2):
                            z_ps = pmf.tile([128, 512], F32, tag="mm")
                            for fc in range(nF):
                                nc.tensor.matmul(
                                    z_ps[:], uT[:, fc, tb * 128:(tb + 1) * 128],
                                    w2gs[fc // 8][:, fc % 8, ncol * 512:(ncol + 1) * 512],
                                    start=(fc == 0),
                                    stop=(fc == nF - 1 and ("bz", l) not in bias_sb))
                            if ("bz", l) in bias_sb:
                                nc.tensor.matmul(
                                    z_ps[:], ones_bf[:, 0:128],
                                    bias_sb[("bz", l)][:, ncol * 512:(ncol + 1) * 512],
                                    start=False, stop=True)
                            nc.vector.scalar_tensor_tensor(
                                xo[:, ncol * 512:(ncol + 1) * 512], z_ps[:], 1.0,
                                xres[:, ncol * 512:(ncol + 1) * 512],
                                ALU.mult, ALU.add)
                        bn_mv(xo[:], s_in[l + 1][:, r, :])
                        nc.sync.dma_start(Xr.ap()[r * 128:(r + 1) * 128, :], xo[:])

        # ---------------- final: emb = sum_t Xp3 / N + z_pool; layernorm ---
        fin = ctx.enter_context(tc.tile_pool(name="fin", bufs=2))
        gt = fin.tile([1, d.D], F32, tag="lnfg", bufs=1)
        nc.sync.dma_start(gt[:], lnfg.ap())
        bt = fin.tile([1, d.D], F32, tag="lnfb", bufs=1)
        nc.sync.dma_start(bt[:], lnfb.ap())
        for b in range(d.B_loc):
            et = fin.tile([1, 2, 512], F32, tag="et")
            nc.sync.dma_start(et[:], Es.ap()[b:b + 1, :, :])
            zt = fin.tile([1, 2, 512], F32, tag="zt")
            nc.sync.dma_start(zt[:], Zs.ap()[b:b + 1, :, :])
            emb = fin.tile([1, d.D], F32, tag="emb")
            for j in range(2):
                nc.vector.scalar_tensor_tensor(
                    emb[:, j * 512:(j + 1) * 512], et[:, j, :], 1.0 / d.N,
                    zt[:, j, :], ALU.mult, ALU.add)
            bns = small.tile([1, 2, 6], F32, tag="fbns")
            for g in range(2):
                nc.vector.bn_stats(bns[:, g, :], emb[:, g * 512:(g + 1) * 512])
            mv = small.tile([1, 2], F32, tag="fmv")
            nc.vector.bn_aggr(mv[:], bns[:])
            lnv = small.tile([1, 1], F32, tag="flnv")
            nc.scalar.activation(lnv[:], mv[:, 1:2], AF.Ln, bias=eps_t[:1, :])
            rstd = small.tile([1, 1], F32, tag="frstd")
            nc.scalar.activation(rstd[:], lnv[:], AF.Exp, scale=-0.5)
            nmr = small.tile([1, 1], F32, tag="fnmr")
            nc.vector.scalar_tensor_tensor(nmr[:], mv[:, 0:1], -1.0, rstd[:],
                                           ALU.mult, ALU.mult)
            nrm = fin.tile([1, d.D], F32, tag="nrm")
            nc.vector.tensor_scalar(nrm[:], emb[:], rstd[:], nmr[:],
                                    ALU.mult, ALU.add)
            ot = fin.tile([1, d.D], F32, tag="ot")
            nc.vector.tensor_mul(ot[:], nrm[:], gt[:])
            nc.vector.tensor_add(ot[:], ot[:], bt[:])
            nc.sync.dma_start(out.ap()[b:b + 1, :], ot[:])

    nc.compile()
    return nc


_CACHE = {}


def _to_bf16(a):
    return np.asarray(a, dtype=np.float32).astype(ml_dtypes.bfloat16)


def prepare_inputs(dims: Dims, n_cores, token_ids, token_emb, pos_emb, ln1_g, ln1_b,
                   Wq, Wk, Pk, Wo, bo, ln2_g, ln2_b, W1, b1, W2, b2, lnf_g, lnf_b):
    d = dims
    token_ids = np.asarray(token_ids)
    token_emb = np.asarray(token_emb, dtype=np.float32)
    pos_emb = np.asarray(pos_emb, dtype=np.float32)

    x_all = token_emb[token_ids[:, :d.N]] + pos_emb[None, :d.N, :]  # [B, N, D]
    B = token_ids.shape[0]
    assert B == n_cores * d.B_loc

    biases = {}
    shared = {}
    for l in range(d.L):
        g1 = np.asarray(ln1_g[l], np.float32)
        b1l = np.asarray(ln1_b[l], np.float32)
        g2 = np.asarray(ln2_g[l], np.float32)
        Wql = np.asarray(Wq[l], np.float32)
        Wkl = np.asarray(Wk[l], np.float32)
        W1l = np.asarray(W1[l], np.float32)
        shared[f"wq{l}"] = _to_bf16(g1[:, None] * Wql)
        shared[f"wk{l}"] = _to_bf16(g1[:, None] * Wkl)
        shared[f"pk{l}"] = _to_bf16(np.asarray(Pk[l])[:d.N])
        shared[f"wo{l}"] = _to_bf16(Wo[l])
        shared[f"w1{l}"] = _to_bf16(g2[:, None] * W1l)
        shared[f"w2{l}"] = _to_bf16(W2[l])

        def nz(v):
            v = np.asarray(v, np.float32)
            return v if np.any(v != 0) else None

        cq = nz(b1l @ Wql)
        bul = nz(np.asarray(ln2_b[l], np.float32) @ W1l + np.asarray(b1[l], np.float32))
        bol = nz(bo[l])
        bzl = nz(b2[l])
        biases[("cq", l)] = _to_bf16(cq) if cq is not None else None
        biases[("bu", l)] = _to_bf16(bul) if bul is not None else None
        biases[("bo", l)] = _to_bf16(bol) if bol is not None else None
        biases[("bz", l)] = _to_bf16(bzl) if bzl is not None else None

    lnf_g_rep = np.asarray(lnf_g, np.float32).reshape(1, d.D).copy()
    lnf_b_rep = np.asarray(lnf_b, np.float32).reshape(1, d.D).copy()

    in_maps = []
    for c in range(n_cores):
        m = dict(shared)
        xc = np.ascontiguousarray(
            x_all[c * d.B_loc:(c + 1) * d.B_loc].reshape(d.R, d.D), dtype=np.float32)
        m["x0"] = xc
        # layer-0 LN stats, laid out [128, n_blocks, 2]
        mean = xc.mean(axis=1).astype(np.float32)       # [R]
        var = xc.var(axis=1).astype(np.float32)         # [R]
        s = np.stack([mean, var], axis=-1).reshape(d.R // 128, 128, 2)
        m["s1_0"] = np.ascontiguousarray(s.transpose(1, 0, 2))
        m["lnfg"] = lnf_g_rep
        m["lnfb"] = lnf_b_rep
        for key, v in biases.items():
            if v is not None:
                m[f"{key[0]}{key[1]}"] = v
        in_maps.append(m)
    return in_maps, biases


def run(dims: Dims, n_cores, inputs, trace=False, tmpdir=None):
    in_maps, biases = prepare_inputs(dims, n_cores, **inputs)
    ck = (dims, n_cores, tuple(sorted(k for k, v in biases.items() if v is not None)))
    if ck not in _CACHE:
        _CACHE[ck] = build(dims, n_cores, biases)
    nc = _CACHE[ck]
    res = run_bass_kernel_spmd(nc, in_maps, list(range(n_cores)), trace=trace,
                               tmpdir=tmpdir)
    outs = np.concatenate([res.results[i]["out"] for i in range(n_cores)], axis=0)
    return outs, res


def kernel(**inputs) -> np.ndarray:
    out, _ = run(Dims(), 8, inputs)
    return out.astype(np.float32)
